# revision 1
# baseline (speedup 1.0000x reference)
"""Trainium2 Bass kernel for 2-layer residual BiLSTM (B=256, T=512, D=U=256).

Strategy (per spec sharding hint, data-parallel over batch):
  - 8 cores, each owns a 32-row batch shard and runs BOTH directions (fw, bw)
    as two independent interleaved streams; layer 0 then layer 1 as two
    sequential phases. h0^T round-trips DRAM between phases and doubles as
    the layer-1 projection input AND the residual addend.
  - "T-layout": gates/units live on SBUF/PSUM partitions, batch on the free
    dim, so each step's new hidden state h^T is produced directly in the
    layout the next step's matmul consumes (no per-step transposes).
  - Per step, z^T = Wx^T x_t + Wh^T h_{t-1} (+bias via ACT) accumulates in
    PSUM: the x-projection is issued as a chunked GEMM (4 steps at a time)
    into the same PSUM banks the recurrent matmuls then accumulate onto.
  - Gate column order is permuted to [g, i, f, o] so tanh(g) and
    sigmoid(i,f) are packed ACT instructions on the recurrence's critical
    path while sigmoid(o) runs off-chain (only needed after tanh(c)).
  - Weights / x / h in fp16 (PE 1 cyc/row + FWL weight loads, ~8x better
    mantissa than bf16), PSUM/gates/c in fp32.

Host side shards/pre-transposes inputs, launches the SPMD kernel on 8
cores, and merges (fw+bw)/2 + untransposes the outputs.
"""

import os

# Persistent JAX/PJRT compilation cache: makes repeat kernel() invocations
# (fresh processes included) skip the multi-minute neuronx-cc compile.
os.environ.setdefault("JAX_COMPILATION_CACHE_DIR", "/tmp/bilstm_jax_cache")

import numpy as np

# Problem shape (hardcoded per harness contract)
B, T, D, U = 256, 512, 256, 256
NCORES = 8
BS = B // NCORES        # batch rows per core (= per stream)
G4 = 4 * U              # 1024 gate columns
NM = G4 // 128          # 8 m-chunks of gate columns
NK = U // 128           # 2 k-chunks of contraction dim
TCP = 4                 # steps per PSUM projection chunk (2 banks / chunk)
TCX = 32                # steps per input ring chunk

# gate column permutation: original order [i f g o] -> ours [g i f o]
_GATE_PERM = np.r_[2 * U:3 * U, 0:U, U:2 * U, 3 * U:4 * U]

_BUILD_CACHE = {}


def _build(T_, dtype="fp16"):
    """Build the SPMD Bass program (same program on all cores)."""
    from contextlib import ExitStack

    import concourse.bacc as bacc
    import concourse.bass as bass
    import concourse.mybir as mybir
    import concourse.tile as tile

    f32 = mybir.dt.float32
    wdt = {"fp32": f32, "bf16": mybir.dt.bfloat16, "fp16": mybir.dt.float16}[dtype]
    AF = mybir.ActivationFunctionType

    nc = bacc.Bacc("TRN2", target_bir_lowering=False, debug=False)

    xT = nc.dram_tensor("xT", [NK, 128, T_, BS], wdt, kind="ExternalInput")
    W = {}
    for d in "fb":
        for l in (0, 1):
            for wch in "xh":
                W[d, l, wch] = nc.dram_tensor(
                    f"W{d}{l}{wch}", [NK, 128, G4], wdt, kind="ExternalInput"
                )
    out_d = {
        d: nc.dram_tensor(f"out_{d}", [T_, 128, NK, BS], f32, kind="ExternalOutput")
        for d in "fb"
    }

    with ExitStack() as ctx:
        tc = ctx.enter_context(tile.TileContext(nc))
        wpool = ctx.enter_context(tc.tile_pool(name="w", bufs=1))
        ring = ctx.enter_context(tc.tile_pool(name="ring", bufs=3))
        state = ctx.enter_context(tc.tile_pool(name="state", bufs=1))
        gates = ctx.enter_context(tc.tile_pool(name="gates", bufs=4))
        outp = ctx.enter_context(tc.tile_pool(name="outp", bufs=6))
        psum = ctx.enter_context(
            tc.tile_pool(name="psum", bufs=2, space=bass.MemorySpace.PSUM)
        )
        dram = ctx.enter_context(
            tc.tile_pool(name="dram", bufs=1, space=bass.MemorySpace.DRAM)
        )

        # --- load weights (all dirs/layers) into SBUF once ---
        wsb = {}
        for d in "fb":
            for l in (0, 1):
                for wch in "xh":
                    t = wpool.tile([128, NK, G4], wdt, tag=f"W{d}{l}{wch}", name=f"W{d}{l}{wch}sb")
                    for k in range(NK):
                        nc.sync.dma_start(t[:, k, :], W[d, l, wch][k])
                    wsb[d, l, wch] = t

        # h0^T interphase scratch (DRAM pool so Tile tracks the RAW dep)
        h0T = {d: dram.tile([T_, 128, NK, BS], wdt, tag=f"h0T{d}", name=f"h0T{d}") for d in "fb"}

        # persistent per-stream state
        hT = {}
        cst = {}
        for d in "fb":
            hT[d] = [
                state.tile([128, NK, BS], wdt, tag=f"hT{d}{i}", name=f"hT{d}{i}")
                for i in range(4)
            ]
            cst[d] = state.tile([128, NK, BS], f32, tag=f"c{d}", name=f"c{d}")

        for phase in (0, 1):
            rsrc = {}  # per (d, k): function t_block -> DRAM AP for ring load
            for d in "fb":
                if phase == 0:
                    rsrc[d] = lambda tb, k, _d=d: xT[k, :, tb:tb + TCX, :]
                else:
                    rsrc[d] = (
                        lambda tb, k, _d=d: h0T[_d][tb:tb + TCX, :, k, :]
                        .rearrange("t p b -> p t b")
                    )
                # reset scan state for this phase (round 0 reads hT[d][0])
                nc.gpsimd.memset(hT[d][0][:], 0.0)
                nc.gpsimd.memset(cst[d][:], 0.0)

            ringt = {}
            zc = {}
            hidx = {d: 0 for d in "fb"}

            for r in range(T_):
                tt = {}
                # --- stage 0 per stream: ring refill + proj chunk + h-MMs ---
                for d in "fb":
                    t = r if d == "f" else T_ - 1 - r
                    tt[d] = t
                    wx = wsb[d, phase, "x"]
                    wh = wsb[d, phase, "h"]

                    # --- input ring refill (every TCX steps) ---
                    if r % TCX == 0:
                        tb = t - (TCX - 1) if d == "b" else t
                        rt = ring.tile([128, NK, TCX, BS], wdt, tag=f"ring{d}")
                        for k in range(NK):
                            nc.sync.dma_start(rt[:, k, :, :], rsrc[d](tb, k))
                        ringt[d] = (rt, tb)

                    rt, tb = ringt[d]

                    # --- projection chunk (every TCP steps) ---
                    if r % TCP == 0:
                        c0 = t - (TCP - 1) if d == "b" else t
                        z = psum.tile([128, NM, TCP, BS], f32, tag=f"z{d}")
                        # start=True zero-marks the WHOLE 2KB psum bank, so
                        # only the first matmul into each bank may carry it.
                        bank_m = NM // 2  # m-chunks per psum bank
                        for m in range(NM):
                            for k in range(NK):
                                nc.tensor.matmul(
                                    z[:, m, :, :],
                                    wx[:, k, m * 128:(m + 1) * 128],
                                    rt[:, k, c0 - tb:c0 - tb + TCP, :],
                                    start=(k == 0 and m % bank_m == 0),
                                    stop=False,
                                    skip_group_check=True,
                                )
                        zc[d] = (z, c0)

                    z, c0 = zc[d]
                    j = t - c0  # step slot inside psum chunk

                    # --- recurrent matmuls (accumulate onto projection) ---
                    last_of_chunk = r % TCP == TCP - 1
                    bank_m = NM // 2
                    for m in range(NM):
                        for k in range(NK):
                            nc.tensor.matmul(
                                z[:, m, j, :],
                                wh[:, k, m * 128:(m + 1) * 128],
                                hT[d][hidx[d] % 4][:, k, :],
                                start=False,
                                stop=(
                                    last_of_chunk
                                    and k == NK - 1
                                    and m % bank_m == bank_m - 1
                                ),
                                skip_group_check=True,
                            )

                # --- stage 1 per stream: gates + cell/hidden update ---
                for d in "fb":
                    t = tt[d]
                    rt, tb = ringt[d]
                    z, c0 = zc[d]
                    j = t - c0

                    # chain-critical: tanh(g), sigmoid(i,f); sigmoid(o) is
                    # only needed after tanh(c) and stays off the chain.
                    tg = gates.tile([128, NK, BS], f32, tag=f"tg{d}")
                    nc.scalar.activation(tg[:], z[:, 0:NK, j, :], AF.Tanh, bias=1.0)
                    sif = gates.tile([128, 2 * NK, BS], f32, tag=f"sif{d}")
                    nc.scalar.activation(
                        sif[:], z[:, NK:3 * NK, j, :], AF.Sigmoid, bias=1.0
                    )

                    t1 = gates.tile([128, NK, BS], f32, tag=f"t1{d}")
                    nc.vector.tensor_mul(t1[:], sif[:, 0:NK, :], tg[:])  # i*g
                    t2 = gates.tile([128, NK, BS], f32, tag=f"t2{d}")
                    nc.vector.tensor_mul(t2[:], sif[:, NK:2 * NK, :], cst[d][:])
                    nc.vector.tensor_add(cst[d][:], t1[:], t2[:])
                    so = gates.tile([128, NK, BS], f32, tag=f"so{d}")
                    nc.scalar.activation(
                        so[:], z[:, 3 * NK:NM, j, :], AF.Sigmoid, bias=1.0
                    )
                    th = gates.tile([128, NK, BS], f32, tag=f"th{d}")
                    nc.scalar.activation(th[:], cst[d][:], AF.Tanh)

                    hn = hT[d][(hidx[d] + 1) % 4]
                    nc.vector.tensor_mul(hn[:], so[:], th[:])
                    if phase == 0:
                        nc.sync.dma_start(
                            h0T[d][t].rearrange("p k b -> p (k b)"),
                            hn.rearrange("p k b -> p (k b)"),
                        )
                    else:
                        ot = outp.tile([128, NK, BS], f32, tag=f"ot{d}")
                        nc.gpsimd.tensor_add(
                            ot[:], hn[:],
                            rt[:, :, t - tb, :].rearrange("p k b -> p k b"),
                        )
                        nc.sync.dma_start(
                            out_d[d][t].rearrange("p k b -> p (k b)"),
                            ot.rearrange("p k b -> p (k b)"),
                        )

                    hidx[d] += 1

    nc.compile()
    return nc


def _prep_inputs(inputs, T_, dtype="fp16"):
    """Host-side shard + layout prep. Returns per-core input maps."""
    import ml_dtypes

    wdt = {"fp32": np.float32, "bf16": ml_dtypes.bfloat16, "fp16": np.float16}[dtype]

    x = np.asarray(inputs["x"], dtype=np.float32)

    wmaps = {}
    for d, dd in (("f", "fw"), ("b", "bw")):
        for l in (0, 1):
            for wch, key in (("x", "Wx"), ("h", "Wh")):
                w = np.asarray(inputs[f"{dd}{l}_{key}"], dtype=np.float32)
                wp = w[:, _GATE_PERM].reshape(NK, 128, G4)
                wmaps[f"W{d}{l}{wch}"] = np.ascontiguousarray(wp).astype(wdt)
            bb = np.asarray(inputs[f"{dd}{l}_b"], dtype=np.float32)
            if not np.allclose(bb, 1.0, atol=0.0):
                raise NotImplementedError(
                    "kernel assumes bias == ones (keras bias_initializer='ones')"
                )

    in_maps = []
    for ci in range(NCORES):
        xs = x[ci * BS:(ci + 1) * BS, :T_, :]          # [BS, T_, D]
        xT = np.ascontiguousarray(xs.transpose(2, 1, 0))  # [D, T_, BS]
        xT = xT.reshape(NK, 128, T_, BS).astype(wdt)
        m = {"xT": xT}
        m.update(wmaps)
        in_maps.append(m)
    return in_maps


def _assemble(results, T_):
    out = np.empty((B, T_, U), dtype=np.float32)
    for ci, res in enumerate(results):
        arr = (res["out_f"] + res["out_b"]) * 0.5       # [T_, 128, NK, BS]
        # out[b, t, k*128 + p] = arr[t, p, k, b]
        out[ci * BS:(ci + 1) * BS] = (
            arr.transpose(3, 0, 2, 1).reshape(BS, T_, U)
        )
    return out


def _setup_jax_cache():
    try:
        import jax

        jax.config.update("jax_compilation_cache_dir",
                          os.environ["JAX_COMPILATION_CACHE_DIR"])
        jax.config.update("jax_persistent_cache_min_compile_time_secs", 1.0)
        jax.config.update("jax_persistent_cache_min_entry_size_bytes", 0)
    except Exception:
        pass


def kernel(**inputs) -> np.ndarray:
    _setup_jax_cache()
    from concourse.bass_utils import run_bass_kernel_spmd

    dtype = "fp16"
    key = (T, dtype)
    if key not in _BUILD_CACHE:
        _BUILD_CACHE[key] = _build(T, dtype)
    nc = _BUILD_CACHE[key]

    in_maps = _prep_inputs(inputs, T, dtype)
    res = run_bass_kernel_spmd(nc, in_maps, core_ids=list(range(NCORES)))
    return _assemble(res.results, T)



# revision 2
# speedup vs baseline: 11.9892x; 11.9892x over previous
"""Trainium2 Bass kernel for 2-layer residual BiLSTM (B=256, T=512, D=U=256).

Strategy v3 (direction-split data parallel + on-core layer pipelining):
  - Cores 0-3 run the FORWARD direction on batch quarters (64 rows each);
    cores 4-7 run BACKWARD on time-reversed inputs (same SPMD program --
    reversal happens host-side, outputs un-reversed on assembly). The fw and
    bw chains only meet at the final average, which the host computes.
  - Each core runs BOTH layers as two interleaved streams: layer 1 lags
    layer 0 by LAG steps and consumes h0 from an SBUF ring (no DRAM
    round-trip, no phase barrier). Layer-1 residual (h1 + h0) reads the same
    ring.
  - Doubling the per-stream batch to 64 (vs 32 in the data-parallel layout)
    doubles the moving columns per recurrent matmul, halving the
    weight-load-bound PE cost, and halves the ACT instruction count per
    step (the per-instruction overhead ~185ns rivals the payload).
  - "T-layout": gate/unit dims on partitions, batch on the free dim; z =
    Wx^T x (+ones bias via ACT) chunked TCP steps at a time into PSUM, with
    recurrent matmuls accumulating on top. Gate column order [g,i,f,o] so
    tanh(g) is one ACT and sigmoid(i,f,o) is one packed N=384 ACT.
  - Weights / x / h in fp16 (PE 1 cyc/row, FWL weight loads), PSUM/c fp32.
"""

import os

os.environ.setdefault("JAX_COMPILATION_CACHE_DIR", "/tmp/bilstm_jax_cache")

import numpy as np

# Problem shape (hardcoded per harness contract)
B, T, D, U = 256, 512, 256, 256
NCORES = 8
BS = B // (NCORES // 2)  # 64 batch rows per core (one direction per core)
G4 = 4 * U               # 1024 gate columns
NM = G4 // 128           # 8 m-chunks of gate columns
NK = U // 128            # 2 k-chunks of contraction dim
TCP = 4                  # steps per PSUM projection chunk (4 banks / layer)
TCX = 32                 # steps per input ring chunk
LAG = 10                 # layer-1 lag (== 2 mod TCP staggers psum refills)
W = 16                   # h0 SBUF ring slots (multiple of TCP, > LAG + TCP)

# gate column permutation: original order [i f g o] -> ours [g i f o]
_GATE_PERM = np.r_[2 * U:3 * U, 0:U, U:2 * U, 3 * U:4 * U]

_BUILD_CACHE = {}


def _build(T_, dtype="fp16"):
    """Build the SPMD Bass program (same program on all cores)."""
    from contextlib import ExitStack

    import concourse.bacc as bacc
    import concourse.bass as bass
    import concourse.mybir as mybir
    import concourse.tile as tile

    f32 = mybir.dt.float32
    wdt = {"fp32": f32, "bf16": mybir.dt.bfloat16, "fp16": mybir.dt.float16}[dtype]
    AF = mybir.ActivationFunctionType

    nc = bacc.Bacc("TRN2", target_bir_lowering=False, debug=False)

    xT = nc.dram_tensor("xT", [NK, 128, T_, BS], wdt, kind="ExternalInput")
    Wd = {}
    for l in (0, 1):
        for wch in "xh":
            Wd[l, wch] = nc.dram_tensor(
                f"W{wch}{l}", [NK, 128, G4], wdt, kind="ExternalInput"
            )
    out_t = nc.dram_tensor("out", [T_, 128, NK, BS], f32, kind="ExternalOutput")

    with ExitStack() as ctx:
        tc = ctx.enter_context(tile.TileContext(nc))
        wpool = ctx.enter_context(tc.tile_pool(name="w", bufs=1))
        ring = ctx.enter_context(tc.tile_pool(name="ring", bufs=3))
        state = ctx.enter_context(tc.tile_pool(name="state", bufs=1))
        gates = ctx.enter_context(tc.tile_pool(name="gates", bufs=3))
        outp = ctx.enter_context(tc.tile_pool(name="outp", bufs=6))
        psum = ctx.enter_context(
            tc.tile_pool(name="psum", bufs=1, space=bass.MemorySpace.PSUM)
        )

        # --- load weights (both layers) into SBUF once ---
        wsb = {}
        for l in (0, 1):
            for wch in "xh":
                t = wpool.tile([128, NK, G4], wdt, tag=f"W{wch}{l}", name=f"W{wch}{l}sb")
                for k in range(NK):
                    nc.sync.dma_start(t[:, k, :], Wd[l, wch][k])
                wsb[l, wch] = t

        # persistent state
        h0r = state.tile([128, NK, W, BS], wdt, tag="h0r", name="h0r")  # L0 h ring
        c0 = state.tile([128, NK, BS], f32, tag="c0", name="c0")
        c1 = state.tile([128, NK, BS], f32, tag="c1", name="c1")
        h1 = [
            state.tile([128, NK, BS], wdt, tag=f"h1_{i}", name=f"h1_{i}")
            for i in range(4)
        ]
        nc.gpsimd.memset(h0r[:, :, W - 1, :], 0.0)
        nc.gpsimd.memset(c0[:], 0.0)
        nc.gpsimd.memset(c1[:], 0.0)
        nc.gpsimd.memset(h1[0][:], 0.0)

        ringt = None   # (tile, base_t) for L0 x ring
        z0 = z1 = None
        hidx1 = 0

        def lstm_chain(sfx, z, j, c, hout, bias_sig=1.0):
            """Per-step gate math: z[:, :, j, :] -> hout (fp16), update c."""
            tg = gates.tile([128, NK, BS], f32, tag=f"tg{sfx}")
            nc.scalar.activation(tg[:], z[:, 0:NK, j, :], AF.Tanh, bias=1.0)
            sio = gates.tile([128, 3 * NK, BS], f32, tag=f"sio{sfx}")
            nc.scalar.activation(
                sio[:], z[:, NK:NM, j, :], AF.Sigmoid, bias=bias_sig
            )
            ig = gates.tile([128, NK, BS], f32, tag=f"ig{sfx}")
            nc.vector.tensor_mul(ig[:], sio[:, 0:NK, :], tg[:])
            fc = gates.tile([128, NK, BS], f32, tag=f"fc{sfx}")
            nc.vector.tensor_mul(fc[:], sio[:, NK:2 * NK, :], c[:])
            nc.vector.tensor_add(c[:], ig[:], fc[:])
            th = gates.tile([128, NK, BS], f32, tag=f"th{sfx}")
            nc.scalar.activation(th[:], c[:], AF.Tanh)
            nc.vector.tensor_mul(hout, sio[:, 2 * NK:3 * NK, :], th[:])

        for r in range(T_ + LAG):
            # ---------------- layer 0 at t0 = r ----------------
            if r < T_:
                t0 = r
                if t0 % TCX == 0:
                    rt = ring.tile([128, NK, TCX, BS], wdt, tag="ring0")
                    for k in range(NK):
                        nc.sync.dma_start(rt[:, k, :, :], xT[k, :, t0:t0 + TCX, :])
                    ringt = (rt, t0)
                rt, tb = ringt

                if t0 % TCP == 0:
                    z0 = psum.tile([128, NM, TCP, BS], f32, tag="z0")
                    for m in range(NM):
                        for k in range(NK):
                            nc.tensor.matmul(
                                z0[:, m, :, :],
                                wsb[0, "x"][:, k, m * 128:(m + 1) * 128],
                                rt[:, k, t0 - tb:t0 - tb + TCP, :],
                                start=(k == 0 and m % 2 == 0),
                                stop=False,
                                skip_group_check=True,
                            )
                j0 = t0 % TCP
                for m in range(NM):
                    for k in range(NK):
                        nc.tensor.matmul(
                            z0[:, m, j0, :],
                            wsb[0, "h"][:, k, m * 128:(m + 1) * 128],
                            h0r[:, k, (t0 - 1) % W, :],
                            start=False,
                            stop=(j0 == TCP - 1 and k == NK - 1 and m % 2 == 1),
                            skip_group_check=True,
                        )

            # ---------------- layer 1 at t1 = r - LAG ----------------
            if r >= LAG:
                t1 = r - LAG
                if t1 % TCP == 0:
                    s = t1 % W
                    z1 = psum.tile([128, NM, TCP, BS], f32, tag="z1")
                    for m in range(NM):
                        for k in range(NK):
                            nc.tensor.matmul(
                                z1[:, m, :, :],
                                wsb[1, "x"][:, k, m * 128:(m + 1) * 128],
                                h0r[:, k, s:s + TCP, :],
                                start=(k == 0 and m % 2 == 0),
                                stop=False,
                                skip_group_check=True,
                            )
                j1 = t1 % TCP
                for m in range(NM):
                    for k in range(NK):
                        nc.tensor.matmul(
                            z1[:, m, j1, :],
                            wsb[1, "h"][:, k, m * 128:(m + 1) * 128],
                            h1[hidx1 % 4][:, k, :],
                            start=False,
                            stop=(j1 == TCP - 1 and k == NK - 1 and m % 2 == 1),
                            skip_group_check=True,
                        )

            # ---------------- gate chains (L0 then L1) ----------------
            if r < T_:
                t0 = r
                lstm_chain("0", z0, t0 % TCP, c0, h0r[:, :, t0 % W, :])

            if r >= LAG:
                t1 = r - LAG
                hn = h1[(hidx1 + 1) % 4]
                lstm_chain("1", z1, t1 % TCP, c1, hn[:])
                ot = outp.tile([128, NK, BS], f32, tag="ot")
                nc.gpsimd.tensor_add(ot[:], hn[:], h0r[:, :, t1 % W, :])
                nc.sync.dma_start(
                    out_t[t1].rearrange("p k b -> p (k b)"),
                    ot.rearrange("p k b -> p (k b)"),
                )
                hidx1 += 1

    nc.compile()
    return nc


def _prep_inputs(inputs, T_, dtype="fp16"):
    """Host-side shard + layout prep. Returns per-core input maps."""
    import ml_dtypes

    wdt = {"fp32": np.float32, "bf16": ml_dtypes.bfloat16, "fp16": np.float16}[dtype]

    x = np.asarray(inputs["x"], dtype=np.float32)

    wmaps = {}  # per direction
    for d, dd in (("f", "fw"), ("b", "bw")):
        m = {}
        for l in (0, 1):
            for wch, key in (("x", "Wx"), ("h", "Wh")):
                w = np.asarray(inputs[f"{dd}{l}_{key}"], dtype=np.float32)
                wp = w[:, _GATE_PERM].reshape(NK, 128, G4)
                m[f"W{wch}{l}"] = np.ascontiguousarray(wp).astype(wdt)
            bb = np.asarray(inputs[f"{dd}{l}_b"], dtype=np.float32)
            if not np.allclose(bb, 1.0, atol=0.0):
                raise NotImplementedError(
                    "kernel assumes bias == ones (keras bias_initializer='ones')"
                )
        wmaps[d] = m

    in_maps = []
    for ci in range(NCORES):
        d = "f" if ci < 4 else "b"
        q = ci % 4
        xs = x[q * BS:(q + 1) * BS, :T_, :]            # [BS, T_, D]
        if d == "b":
            xs = xs[:, ::-1, :]                        # time-reverse for bw
        xTc = np.ascontiguousarray(xs.transpose(2, 1, 0))  # [D, T_, BS]
        xTc = xTc.reshape(NK, 128, T_, BS).astype(wdt)
        m = {"xT": xTc}
        m.update(wmaps[d])
        in_maps.append(m)
    return in_maps


def _assemble(results, T_):
    out = np.empty((B, T_, U), dtype=np.float32)
    for q in range(4):
        af = results[q]["out"]          # [T_, 128, NK, BS] fw
        ab = results[q + 4]["out"]      # [T_, 128, NK, BS] bw (reversed time)
        # out[b, t, k*128 + p] = arr[t, p, k, b]
        f = af.transpose(3, 0, 2, 1).reshape(BS, T_, U)
        bwd = ab[::-1].transpose(3, 0, 2, 1).reshape(BS, T_, U)
        out[q * BS:(q + 1) * BS] = (f + bwd) * 0.5
    return out


def _setup_jax_cache():
    try:
        import jax

        jax.config.update("jax_compilation_cache_dir",
                          os.environ["JAX_COMPILATION_CACHE_DIR"])
        jax.config.update("jax_persistent_cache_min_compile_time_secs", 1.0)
        jax.config.update("jax_persistent_cache_min_entry_size_bytes", 0)
    except Exception:
        pass


def kernel(**inputs) -> np.ndarray:
    _setup_jax_cache()
    from concourse.bass_utils import run_bass_kernel_spmd

    dtype = "fp16"
    key = (T, dtype)
    if key not in _BUILD_CACHE:
        _BUILD_CACHE[key] = _build(T, dtype)
    nc = _BUILD_CACHE[key]

    in_maps = _prep_inputs(inputs, T, dtype)
    res = run_bass_kernel_spmd(nc, in_maps, core_ids=list(range(NCORES)))
    return _assemble(res.results, T)
